# revision 17
# baseline (speedup 1.0000x reference)
"""Trainium2 Bass kernel for the delta-rule memory recurrence (DeltaNet-style).

Full-input contract: kernel(memory, key, value) -> final memory, all np.ndarray,
shapes (16,256,256), (16,4096,256), (16,4096,256) -> (16,256,256) float32.

Strategy: pure data-parallel over batch (2 batches per NeuronCore x 8 cores).
Per batch the sequential recurrence

    kn   = k_t / ||k_t||
    M   <- M - (1.1 * M kn - 0.1 * v_t) kn^T

is reformulated chunkwise (C=128 steps per chunk) via the WY / UT transform:

    A  = Kn Kn^T                      (C x C Gram of normalized keys)
    L  = 1.1 * strict_lower(A)
    Tinv = (I + L)^{-1}               (unit lower triangular inverse)
    H  = Tinv @ (-1.1 * Kn Mt + 0.1 * V)
    Mt <- Mt + Kn^T H                 (Mt = M^T state, (DK, DV))

(I+L)^{-1} is computed exactly with the nilpotent factorization
(I-L)(I+L^2)(I+L^4)(I+L^8)  [L^16 and beyond are numerically zero here].
Inversion machinery runs in fp16 matmuls (full PE rate, 10-bit mantissa),
state-path matmuls run as float32r (full rate at N>=256).
"""

import numpy as np

import concourse.bass as bass
import concourse.mybir as mybir
import concourse.tile as tile
from concourse.bass import ts
from concourse.bass_utils import run_bass_kernel_spmd
from concourse.masks import make_identity

F32 = mybir.dt.float32
F32R = mybir.dt.float32r
F16 = mybir.dt.float16
AOP = mybir.AluOpType
AFT = mybir.ActivationFunctionType

B, S, DK, DV = 16, 4096, 256, 256
NCORES = 8
BLOC = B // NCORES          # batches per core
C = 128                     # chunk length
LR = 0.1
AC = 1.0 + LR               # 1.1
NLEV = 3                    # squaring levels: (I-L)(I+L^2)(I+L^4)(I+L^8)


def _split_waits(nc, max_waits=1):
    """walrus codegen on this toolchain encodes at most one semaphore wait per
    instruction; hoist excess waits onto same-engine NoOps placed just before."""
    n_split = 0
    for f in nc.m.functions:
        for bb in f.blocks:
            insts = bb.instructions
            out = []
            for inst in insts:
                si = getattr(inst, "sync_info", None)
                w = list(si.on_wait) if (si and si.on_wait) else []
                k = 0
                while len(w) > max_waits:
                    head, w = w[:max_waits], w[max_waits:]
                    out.append(mybir.InstNoOp(
                        name=f"{inst.name}-wsplit{k}",
                        engine=inst.engine,
                        sync_info=mybir.SyncInfo(on_wait=head, on_update=[]),
                    ))
                    n_split += 1
                    k += 1
                if k:
                    inst.sync_info = mybir.SyncInfo(
                        on_wait=w, on_update=list(si.on_update or [])
                    )
                out.append(inst)
            bb.instructions = out
    return n_split


def build_nc(s_loc=S, state_mm_dtype=F32R, split=True):
    nch = s_loc // C
    nc = bass.Bass()
    memT = nc.declare_dram_parameter("memT", [BLOC, DK, DV], F32, isOutput=False)
    key_d = nc.declare_dram_parameter("key", [BLOC, s_loc, DK], F32, isOutput=False)
    val_d = nc.declare_dram_parameter("value", [BLOC, s_loc, DV], F32, isOutput=False)
    outT = nc.declare_dram_parameter("outT", [BLOC, DK, DV], F32, isOutput=True)

    SMM = state_mm_dtype  # state-path matmul tiles (float32r: full-rate fp32-ish mm)

    with tile.TileContext(nc) as tc:
        with (
            tc.tile_pool(name="consts", bufs=1) as consts,
            tc.tile_pool(name="kv", bufs=4) as kv,
            tc.tile_pool(name="norm", bufs=4) as normp,
            tc.tile_pool(name="kt", bufs=3) as ktp,
            tc.tile_pool(name="inv", bufs=3) as invp,
            tc.tile_pool(name="state", bufs=3) as statep,
            tc.tile_pool(name="mt", bufs=3) as mtp,
            tc.tile_pool(name="ps_inv", bufs=4, space="PSUM") as ps_inv,
            tc.tile_pool(name="ps_state", bufs=2, space="PSUM") as ps_state,
            tc.tile_pool(name="ps_mt0", bufs=1, space="PSUM") as ps_mt0,
            tc.tile_pool(name="ps_mt1", bufs=1, space="PSUM") as ps_mt1,
        ):
            ident32 = consts.tile([128, 128], F32, tag="ident32")
            make_identity(nc, ident32)
            ident16 = consts.tile([128, 128], F16, tag="ident16")
            make_identity(nc, ident16)
            # paired identity (both halves) for G0 = I + LTn
            i2_16 = consts.tile([128, 2, 128], F16, tag="i2_16")
            nc.gpsimd.memset(i2_16, 0.0)
            nc.gpsimd.affine_select(
                out=i2_16, in_=i2_16, compare_op=AOP.not_equal, fill=1.0,
                base=0, pattern=[[0, 2], [-1, 128]], channel_multiplier=1,
            )

            # state Mt (= M^T) per batch lives in PSUM and accumulates the
            # per-chunk updates; an SBUF f32r copy is refreshed each chunk.
            # Initial value injected via exact fp32 identity-matmul.
            mt = []
            mt_ps = []
            for b, pool in ((0, ps_mt0), (1, ps_mt1)):
                t0 = mtp.tile([128, 2, DV], F32, tag=f"mt0f{b}")
                nc.sync.dma_start(
                    out=t0, in_=memT[b].rearrange("(j p) v -> p j v", p=128)
                )
                ps = pool.tile([128, 2, DV], F32, tag=f"mtps{b}")
                # one matmul over the whole [128, 512] bank: a second
                # start=True would clear the first slice's has_written bits
                nc.tensor.matmul(ps.rearrange("p j v -> p (j v)"), ident32,
                                 t0.rearrange("p j v -> p (j v)"),
                                 start=True, stop=False,
                                 skip_group_check=True)
                t = mtp.tile([128, 2, DV], SMM, tag=f"mt{b}")
                nc.vector.tensor_copy(t, ps)
                mt.append(t)
                mt_ps.append(ps)

            def emit_precomp(c):
                Kt, Vt, Kn = [], [], []
                for b in range(BLOC):
                    k = kv.tile([128, DK], F32, tag=f"k{b}")
                    nc.sync.dma_start(out=k, in_=key_d[b, c * C:(c + 1) * C, :])
                    v = kv.tile([128, DV], F32, tag=f"v{b}")
                    nc.sync.dma_start(out=v, in_=val_d[b, c * C:(c + 1) * C, :])
                    Kt.append(k)
                    Vt.append(v)
                # normalization: ssq/rn for both batches share [128, 2] tiles
                ssq = normp.tile([128, 2], F32, tag="ssq")
                for b in range(BLOC):
                    scr = normp.tile([128, DK], F32, tag="scr")
                    nc.scalar.activation(out=scr, in_=Kt[b], func=AFT.Square,
                                         accum_out=ssq[:, b:b + 1])
                nrm = normp.tile([128, 2], F32, tag="nrm")
                nc.scalar.activation(nrm, ssq, AFT.Sqrt)
                rn = normp.tile([128, 2], F32, tag="rn")
                nc.vector.reciprocal(rn, nrm)
                for b in range(BLOC):
                    kn = normp.tile([128, DK], SMM, tag=f"kn{b}")
                    nc.vector.tensor_scalar_mul(kn, Kt[b], rn[:, b:b + 1])
                    Kn.append(kn)

                # transposes of Kn -> KnTs (f32r, used by A-mm and Y-mm)
                KnTs = [None] * 2
                for j in range(2):
                    tp = ps_inv.tile([128, 2, 128], F32, tag="inv")
                    for b in range(BLOC):
                        nc.tensor.transpose(
                            tp[:, b, :], Kn[b][:, ts(j, 128)].bitcast(F32),
                            ident32)
                    s32 = ktp.tile([128, 2, 128], SMM, tag=f"knts{j}")
                    nc.scalar.copy(s32, tp)
                    KnTs[j] = s32

                # A = Kn Kn^T (f32r matmul; both batches share the psum tile)
                a_ps = ps_inv.tile([128, 2, 128], F32, tag="inv")
                for b in range(BLOC):
                    for j in range(2):
                        nc.tensor.matmul(
                            a_ps[:, b, :], KnTs[j][:, b, :], KnTs[j][:, b, :],
                            start=(j == 0), stop=(j == 1),
                        )
                a_neg = invp.tile([128, 2, 128], F16, tag="a_neg")
                nc.scalar.mul(a_neg, a_ps, -AC)
                # Ln = -L = strict_lower(a_neg); LTn = -L^T = strict_upper(a_neg)
                ln = invp.tile([128, 2, 128], F16, tag="ln")
                nc.gpsimd.affine_select(
                    out=ln, in_=a_neg, compare_op=AOP.is_gt, fill=0.0,
                    base=0, pattern=[[0, 2], [-1, 128]], channel_multiplier=1,
                )
                ltn = invp.tile([128, 2, 128], F16, tag="ltn")
                nc.gpsimd.affine_select(
                    out=ltn, in_=a_neg, compare_op=AOP.is_gt, fill=0.0,
                    base=0, pattern=[[0, 2], [1, 128]], channel_multiplier=-1,
                )

                # power chain; L/LT pairs packed into one [128, 2, 256] psum
                def pow_pair(lhsT_l, rhs_l, lhsT_lt, rhs_lt, tag, eng):
                    ps = ps_inv.tile([128, 2, 256], F32, tag="inv")
                    for b in range(BLOC):
                        nc.tensor.matmul(ps[:, b, 0:128],
                                         lhsT_l[:, b, :], rhs_l[:, b, :])
                        nc.tensor.matmul(ps[:, b, 128:256],
                                         lhsT_lt[:, b, :], rhs_lt[:, b, :])
                    sb = invp.tile([128, 2, 256], F16, tag=tag)
                    if eng == "v":
                        nc.vector.tensor_copy(sb, ps)
                    else:
                        nc.scalar.copy(sb, ps)
                    return sb[:, :, 0:128], sb[:, :, 128:256]

                # L2 = LTn^T@Ln, LT2 = Ln^T@LTn ; L4 = LT2^T... ; L8
                l2, lt2 = pow_pair(ltn, ln, ln, ltn, "p2", "v")
                l4, lt4 = pow_pair(lt2, l2, l2, lt2, "p4", "s")
                p8 = ps_inv.tile([128, 2, 128], F32, tag="inv")
                for b in range(BLOC):
                    nc.tensor.matmul(p8[:, b, :], lt4[:, b, :], l4[:, b, :])
                l8 = invp.tile([128, 2, 128], F16, tag="p8")
                nc.vector.tensor_copy(l8, p8)

                # G chain: G0 = I + LTn; G <- (I + LT^{2^i}) G via psum inject
                g = invp.tile([128, 2, 128], F16, tag="g0")
                nc.vector.tensor_add(g, i2_16, ltn)
                for i, lp in enumerate((l2, l4, l8)):
                    gp = ps_inv.tile([128, 2, 128], F32, tag="inv")
                    gn = invp.tile([128, 2, 128], F16, tag=f"g{i + 1}")
                    if i == 1:
                        # G' = G + LT^4 G : matmul + DVE add (no inject)
                        for b in range(BLOC):
                            nc.tensor.matmul(gp[:, b, :], lp[:, b, :], g[:, b, :])
                        nc.vector.tensor_add(gn, g, gp)
                    else:
                        for b in range(BLOC):
                            nc.tensor.matmul(gp[:, b, :], lp[:, b, :], g[:, b, :],
                                             start=True, stop=False)
                            nc.tensor.matmul(gp[:, b, :], ident16, g[:, b, :],
                                             start=False, stop=True)
                        nc.scalar.copy(gn, gp)
                    g = gn

                return dict(Kn=Kn, Vt=Vt, KnTs=KnTs, g=g, c=c)

            def emit_state(art):
                Kn, Vt, KnTs, g = art["Kn"], art["Vt"], art["KnTs"], art["g"]
                for b in range(BLOC):
                    y_ps = ps_state.tile([128, DV], F32, tag="st")
                    for j in range(2):
                        nc.tensor.matmul(
                            y_ps, KnTs[j][:, b, :], mt[b][:, j, :],
                            start=(j == 0), stop=(j == 1),
                        )
                    # R' = 10*R = -11 Kn Mt + V  (fp16); the 0.1 folds into H
                    rh = statep.tile([128, DV], F16, tag=f"rh{b}")
                    nc.vector.scalar_tensor_tensor(
                        out=rh, in0=y_ps, scalar=-10.0 * AC, in1=Vt[b],
                        op0=AOP.mult, op1=AOP.add,
                    )
                    h_ps = ps_state.tile([128, DV], F32, tag="st")
                    nc.tensor.matmul(h_ps, g[:, b, :], rh)
                    h_sb = statep.tile([128, DV], SMM, tag=f"hs{b}")
                    nc.vector.tensor_scalar_mul(h_sb, h_ps, LR)  # H = 0.1 Tinv R'
                    last = art["c"] == nch - 1
                    for j in range(2):
                        nc.tensor.matmul(
                            mt_ps[b][:, j, :], Kn[b][:, ts(j, 128)], h_sb,
                            start=False, stop=last, skip_group_check=True,
                        )
                    mt_new = mtp.tile([128, 2, DV], SMM, tag=f"mt{b}")
                    nc.vector.tensor_copy(mt_new, mt_ps[b])
                    mt[b] = mt_new

            # software pipeline: chunk c+1's state-independent precompute is
            # emitted before chunk c's state path so the PE always has
            # independent work while psum->sbuf copies drain.
            art = emit_precomp(0)
            for c in range(nch):
                nxt = emit_precomp(c + 1) if c + 1 < nch else None
                emit_state(art)
                art = nxt

            for b in range(BLOC):
                nc.sync.dma_start(
                    out=outT[b].rearrange("(j p) v -> p j v", p=128),
                    in_=mt[b].bitcast(F32),
                )
    if split:
        _split_waits(nc)
    return nc


_NC_CACHE = {}

# test-harness hooks (the grading harness just calls kernel())
TRACE = False
LAST_RESULT = None


def _get_nc(s_loc=S):
    if s_loc not in _NC_CACHE:
        _NC_CACHE[s_loc] = build_nc(s_loc)
    return _NC_CACHE[s_loc]


def kernel(memory, key, value):
    global LAST_RESULT
    memory = np.ascontiguousarray(np.asarray(memory), dtype=np.float32)
    key = np.ascontiguousarray(np.asarray(key), dtype=np.float32)
    value = np.ascontiguousarray(np.asarray(value), dtype=np.float32)
    s_loc = key.shape[1]
    nc = _get_nc(s_loc)
    memT = np.ascontiguousarray(memory.transpose(0, 2, 1))
    in_maps = []
    for i in range(NCORES):
        sl = slice(i * BLOC, (i + 1) * BLOC)
        in_maps.append({
            "memT": memT[sl],
            "key": np.ascontiguousarray(key[sl]),
            "value": np.ascontiguousarray(value[sl]),
        })
    res = run_bass_kernel_spmd(nc, in_maps, list(range(NCORES)), trace=TRACE)
    LAST_RESULT = res
    outs = [res.results[i]["outT"] for i in range(NCORES)]
    out = np.concatenate(outs, axis=0)          # (16, DK, DV) = M^T
    return np.ascontiguousarray(out.transpose(0, 2, 1))


# revision 18
# speedup vs baseline: 1.0325x; 1.0325x over previous
"""Trainium2 Bass kernel for the delta-rule memory recurrence (DeltaNet-style).

Full-input contract: kernel(memory, key, value) -> final memory, all np.ndarray,
shapes (16,256,256), (16,4096,256), (16,4096,256) -> (16,256,256) float32.

Strategy: pure data-parallel over batch (2 batches per NeuronCore x 8 cores).
Per batch the sequential recurrence

    kn   = k_t / ||k_t||
    M   <- M - (1.1 * M kn - 0.1 * v_t) kn^T

is reformulated chunkwise (C=128 steps per chunk) via the WY / UT transform:

    A  = Kn Kn^T                      (C x C Gram of normalized keys)
    L  = 1.1 * strict_lower(A)
    Tinv = (I + L)^{-1}               (unit lower triangular inverse)
    H  = Tinv @ (-1.1 * Kn Mt + 0.1 * V)
    Mt <- Mt + Kn^T H                 (Mt = M^T state, (DK, DV))

(I+L)^{-1} is computed exactly with the nilpotent factorization
(I-L)(I+L^2)(I+L^4)(I+L^8)  [L^16 and beyond are numerically zero here].
Inversion machinery runs in fp16 matmuls (full PE rate, 10-bit mantissa),
state-path matmuls run as float32r (full rate at N>=256).
"""

import numpy as np

import concourse.bass as bass
import concourse.mybir as mybir
import concourse.tile as tile
from concourse.bass import ts
from concourse.bass_utils import run_bass_kernel_spmd
from concourse.masks import make_identity

F32 = mybir.dt.float32
F32R = mybir.dt.float32r
F16 = mybir.dt.float16
AOP = mybir.AluOpType
AFT = mybir.ActivationFunctionType

B, S, DK, DV = 16, 4096, 256, 256
NCORES = 8
BLOC = B // NCORES          # batches per core
C = 128                     # chunk length
LR = 0.1
AC = 1.0 + LR               # 1.1
NLEV = 3                    # squaring levels: (I-L)(I+L^2)(I+L^4)(I+L^8)


def _split_waits(nc, max_waits=1):
    """walrus codegen on this toolchain encodes at most one semaphore wait per
    instruction; hoist excess waits onto same-engine NoOps placed just before."""
    n_split = 0
    for f in nc.m.functions:
        for bb in f.blocks:
            insts = bb.instructions
            out = []
            for inst in insts:
                si = getattr(inst, "sync_info", None)
                w = list(si.on_wait) if (si and si.on_wait) else []
                k = 0
                while len(w) > max_waits:
                    head, w = w[:max_waits], w[max_waits:]
                    out.append(mybir.InstNoOp(
                        name=f"{inst.name}-wsplit{k}",
                        engine=inst.engine,
                        sync_info=mybir.SyncInfo(on_wait=head, on_update=[]),
                    ))
                    n_split += 1
                    k += 1
                if k:
                    inst.sync_info = mybir.SyncInfo(
                        on_wait=w, on_update=list(si.on_update or [])
                    )
                out.append(inst)
            bb.instructions = out
    return n_split


def build_nc(s_loc=S, state_mm_dtype=F32R, split=True):
    nch = s_loc // C
    nc = bass.Bass()
    memT = nc.declare_dram_parameter("memT", [BLOC, DK, DV], F32, isOutput=False)
    key_d = nc.declare_dram_parameter("key", [BLOC, s_loc, DK], F32, isOutput=False)
    val_d = nc.declare_dram_parameter("value", [BLOC, s_loc, DV], F32, isOutput=False)
    outT = nc.declare_dram_parameter("outT", [BLOC, DK, DV], F32, isOutput=True)

    SMM = state_mm_dtype  # state-path matmul tiles (float32r: full-rate fp32-ish mm)

    with tile.TileContext(nc) as tc:
        with (
            tc.tile_pool(name="consts", bufs=1) as consts,
            tc.tile_pool(name="kv", bufs=4) as kv,
            tc.tile_pool(name="norm", bufs=4) as normp,
            tc.tile_pool(name="kt", bufs=3) as ktp,
            tc.tile_pool(name="inv", bufs=3) as invp,
            tc.tile_pool(name="state", bufs=3) as statep,
            tc.tile_pool(name="mt", bufs=3) as mtp,
            tc.tile_pool(name="ps_inv", bufs=3, space="PSUM") as ps_inv,
            tc.tile_pool(name="ps_state", bufs=3, space="PSUM") as ps_state,
            tc.tile_pool(name="ps_mt0", bufs=1, space="PSUM") as ps_mt0,
            tc.tile_pool(name="ps_mt1", bufs=1, space="PSUM") as ps_mt1,
        ):
            ident32 = consts.tile([128, 128], F32, tag="ident32")
            make_identity(nc, ident32)
            ident16 = consts.tile([128, 128], F16, tag="ident16")
            make_identity(nc, ident16)
            # paired identity (both halves) for G0 = I + LTn
            i2_16 = consts.tile([128, 2, 128], F16, tag="i2_16")
            nc.gpsimd.memset(i2_16, 0.0)
            nc.gpsimd.affine_select(
                out=i2_16, in_=i2_16, compare_op=AOP.not_equal, fill=1.0,
                base=0, pattern=[[0, 2], [-1, 128]], channel_multiplier=1,
            )

            # state Mt (= M^T) per batch lives in PSUM and accumulates the
            # per-chunk updates; an SBUF f32r copy is refreshed each chunk.
            # Initial value injected via exact fp32 identity-matmul.
            mt = []
            mt_ps = []
            for b, pool in ((0, ps_mt0), (1, ps_mt1)):
                t0 = mtp.tile([128, 2, DV], F32, tag=f"mt0f{b}")
                nc.sync.dma_start(
                    out=t0, in_=memT[b].rearrange("(j p) v -> p j v", p=128)
                )
                ps = pool.tile([128, 2, DV], F32, tag=f"mtps{b}")
                # one matmul over the whole [128, 512] bank: a second
                # start=True would clear the first slice's has_written bits
                nc.tensor.matmul(ps.rearrange("p j v -> p (j v)"), ident32,
                                 t0.rearrange("p j v -> p (j v)"),
                                 start=True, stop=False,
                                 skip_group_check=True)
                t = mtp.tile([128, 2, DV], SMM, tag=f"mt{b}")
                nc.vector.tensor_copy(t, ps)
                mt.append(t)
                mt_ps.append(ps)

            def emit_precomp(c):
                Kt, Vt, Kn = [], [], []
                for b in range(BLOC):
                    k = kv.tile([128, DK], F32, tag=f"k{b}")
                    nc.sync.dma_start(out=k, in_=key_d[b, c * C:(c + 1) * C, :])
                    v = kv.tile([128, DV], F32, tag=f"v{b}")
                    nc.sync.dma_start(out=v, in_=val_d[b, c * C:(c + 1) * C, :])
                    Kt.append(k)
                    Vt.append(v)
                # normalization: ssq/rn for both batches share [128, 2] tiles
                ssq = normp.tile([128, 2], F32, tag="ssq")
                for b in range(BLOC):
                    scr = normp.tile([128, DK], F32, tag="scr")
                    nc.scalar.activation(out=scr, in_=Kt[b], func=AFT.Square,
                                         accum_out=ssq[:, b:b + 1])
                nrm = normp.tile([128, 2], F32, tag="nrm")
                nc.scalar.activation(nrm, ssq, AFT.Sqrt)
                rn = normp.tile([128, 2], F32, tag="rn")
                nc.vector.reciprocal(rn, nrm)
                for b in range(BLOC):
                    kn = normp.tile([128, DK], SMM, tag=f"kn{b}")
                    nc.vector.tensor_scalar_mul(kn, Kt[b], rn[:, b:b + 1])
                    Kn.append(kn)

                # transposes of Kn -> KnTs (f32r, used by A-mm and Y-mm)
                KnTs = [None] * 2
                for j in range(2):
                    tp = ps_inv.tile([128, 2, 128], F32, tag="inv")
                    for b in range(BLOC):
                        nc.tensor.transpose(
                            tp[:, b, :], Kn[b][:, ts(j, 128)].bitcast(F32),
                            ident32)
                    s32 = ktp.tile([128, 2, 128], SMM, tag=f"knts{j}")
                    nc.scalar.copy(s32, tp)
                    KnTs[j] = s32

                # A = Kn Kn^T (f32r matmul; both batches share the psum tile)
                a_ps = ps_inv.tile([128, 2, 128], F32, tag="inv")
                for b in range(BLOC):
                    for j in range(2):
                        nc.tensor.matmul(
                            a_ps[:, b, :], KnTs[j][:, b, :], KnTs[j][:, b, :],
                            start=(j == 0), stop=(j == 1),
                        )
                a_neg = invp.tile([128, 2, 128], F16, tag="a_neg")
                nc.scalar.mul(a_neg, a_ps, -AC)
                # Ln = -L = strict_lower(a_neg); LTn = -L^T = strict_upper(a_neg)
                ln = invp.tile([128, 2, 128], F16, tag="ln")
                nc.gpsimd.affine_select(
                    out=ln, in_=a_neg, compare_op=AOP.is_gt, fill=0.0,
                    base=0, pattern=[[0, 2], [-1, 128]], channel_multiplier=1,
                )
                ltn = invp.tile([128, 2, 128], F16, tag="ltn")
                nc.gpsimd.affine_select(
                    out=ltn, in_=a_neg, compare_op=AOP.is_gt, fill=0.0,
                    base=0, pattern=[[0, 2], [1, 128]], channel_multiplier=-1,
                )

                # power chain; L/LT pairs packed into one [128, 2, 256] psum
                def pow_pair(lhsT_l, rhs_l, lhsT_lt, rhs_lt, tag, eng):
                    ps = ps_inv.tile([128, 2, 256], F32, tag="inv")
                    for b in range(BLOC):
                        nc.tensor.matmul(ps[:, b, 0:128],
                                         lhsT_l[:, b, :], rhs_l[:, b, :])
                        nc.tensor.matmul(ps[:, b, 128:256],
                                         lhsT_lt[:, b, :], rhs_lt[:, b, :])
                    sb = invp.tile([128, 2, 256], F16, tag=tag)
                    if eng == "v":
                        nc.vector.tensor_copy(sb, ps)
                    else:
                        nc.scalar.copy(sb, ps)
                    return sb[:, :, 0:128], sb[:, :, 128:256]

                # L2 = LTn^T@Ln, LT2 = Ln^T@LTn ; L4 = LT2^T... ; L8
                l2, lt2 = pow_pair(ltn, ln, ln, ltn, "p2", "v")
                l4, lt4 = pow_pair(lt2, l2, l2, lt2, "p4", "s")
                p8 = ps_inv.tile([128, 2, 128], F32, tag="inv")
                for b in range(BLOC):
                    nc.tensor.matmul(p8[:, b, :], lt4[:, b, :], l4[:, b, :])
                l8 = invp.tile([128, 2, 128], F16, tag="p8")
                nc.vector.tensor_copy(l8, p8)

                # G chain: G0 = I + LTn; G <- (I + LT^{2^i}) G via psum inject
                g = invp.tile([128, 2, 128], F16, tag="g0")
                nc.vector.tensor_add(g, i2_16, ltn)
                for i, lp in enumerate((l2, l4, l8)):
                    gp = ps_inv.tile([128, 2, 128], F32, tag="inv")
                    gn = invp.tile([128, 2, 128], F16, tag=f"g{i + 1}")
                    if i == 1:
                        # G' = G + LT^4 G : matmul + DVE add (no inject)
                        for b in range(BLOC):
                            nc.tensor.matmul(gp[:, b, :], lp[:, b, :], g[:, b, :])
                        nc.vector.tensor_add(gn, g, gp)
                    else:
                        for b in range(BLOC):
                            nc.tensor.matmul(gp[:, b, :], lp[:, b, :], g[:, b, :],
                                             start=True, stop=False)
                            nc.tensor.matmul(gp[:, b, :], ident16, g[:, b, :],
                                             start=False, stop=True)
                        nc.scalar.copy(gn, gp)
                    g = gn

                return dict(Kn=Kn, Vt=Vt, KnTs=KnTs, g=g, c=c)

            def emit_state(art):
                Kn, Vt, KnTs, g = art["Kn"], art["Vt"], art["KnTs"], art["g"]
                last = art["c"] == nch - 1
                y_ps, rh, h_ps, h_sb = [], [], [], []
                for b in range(BLOC):
                    y = ps_state.tile([128, DV], F32, tag="st")
                    for j in range(2):
                        nc.tensor.matmul(
                            y, KnTs[j][:, b, :], mt[b][:, j, :],
                            start=(j == 0), stop=(j == 1),
                        )
                    y_ps.append(y)
                for b in range(BLOC):
                    # R' = 10*R = -11 Kn Mt + V  (fp16); the 0.1 folds into H
                    r = statep.tile([128, DV], F16, tag=f"rh{b}")
                    nc.vector.scalar_tensor_tensor(
                        out=r, in0=y_ps[b], scalar=-10.0 * AC, in1=Vt[b],
                        op0=AOP.mult, op1=AOP.add,
                    )
                    rh.append(r)
                for b in range(BLOC):
                    h = ps_state.tile([128, DV], F32, tag="st")
                    nc.tensor.matmul(h, g[:, b, :], rh[b])
                    h_ps.append(h)
                for b in range(BLOC):
                    h = statep.tile([128, DV], SMM, tag=f"hs{b}")
                    nc.scalar.mul(h, h_ps[b], LR)      # H = 0.1 * Tinv R'
                    h_sb.append(h)
                for b in range(BLOC):
                    for j in range(2):
                        nc.tensor.matmul(
                            mt_ps[b][:, j, :], Kn[b][:, ts(j, 128)], h_sb[b],
                            start=False, stop=last, skip_group_check=True,
                        )
                for b in range(BLOC):
                    mt_new = mtp.tile([128, 2, DV], SMM, tag=f"mt{b}")
                    nc.vector.tensor_copy(mt_new, mt_ps[b])
                    mt[b] = mt_new

            # software pipeline: chunk c+1's state-independent precompute is
            # emitted before chunk c's state path so the PE always has
            # independent work while psum->sbuf copies drain.
            arts = [emit_precomp(0), emit_precomp(1) if nch > 1 else None]
            for c in range(nch):
                nxt = emit_precomp(c + 2) if c + 2 < nch else None
                emit_state(arts[0])
                arts = [arts[1], nxt]

            for b in range(BLOC):
                nc.sync.dma_start(
                    out=outT[b].rearrange("(j p) v -> p j v", p=128),
                    in_=mt[b].bitcast(F32),
                )
    if split:
        _split_waits(nc)
    return nc


_NC_CACHE = {}

# test-harness hooks (the grading harness just calls kernel())
TRACE = False
LAST_RESULT = None


def _get_nc(s_loc=S):
    if s_loc not in _NC_CACHE:
        _NC_CACHE[s_loc] = build_nc(s_loc)
    return _NC_CACHE[s_loc]


def kernel(memory, key, value):
    global LAST_RESULT
    memory = np.ascontiguousarray(np.asarray(memory), dtype=np.float32)
    key = np.ascontiguousarray(np.asarray(key), dtype=np.float32)
    value = np.ascontiguousarray(np.asarray(value), dtype=np.float32)
    s_loc = key.shape[1]
    nc = _get_nc(s_loc)
    memT = np.ascontiguousarray(memory.transpose(0, 2, 1))
    in_maps = []
    for i in range(NCORES):
        sl = slice(i * BLOC, (i + 1) * BLOC)
        in_maps.append({
            "memT": memT[sl],
            "key": np.ascontiguousarray(key[sl]),
            "value": np.ascontiguousarray(value[sl]),
        })
    res = run_bass_kernel_spmd(nc, in_maps, list(range(NCORES)), trace=TRACE)
    LAST_RESULT = res
    outs = [res.results[i]["outT"] for i in range(NCORES)]
    out = np.concatenate(outs, axis=0)          # (16, DK, DV) = M^T
    return np.ascontiguousarray(out.transpose(0, 2, 1))


# revision 19
# speedup vs baseline: 1.0530x; 1.0199x over previous
"""Trainium2 Bass kernel for the delta-rule memory recurrence (DeltaNet-style).

Full-input contract: kernel(memory, key, value) -> final memory, all np.ndarray,
shapes (16,256,256), (16,4096,256), (16,4096,256) -> (16,256,256) float32.

Strategy: pure data-parallel over batch (2 batches per NeuronCore x 8 cores).
Per batch the sequential recurrence

    kn   = k_t / ||k_t||
    M   <- M - (1.1 * M kn - 0.1 * v_t) kn^T

is reformulated chunkwise (C=128 steps per chunk) via the WY / UT transform:

    A  = Kn Kn^T                      (C x C Gram of normalized keys)
    L  = 1.1 * strict_lower(A)
    Tinv = (I + L)^{-1}               (unit lower triangular inverse)
    H  = Tinv @ (-1.1 * Kn Mt + 0.1 * V)
    Mt <- Mt + Kn^T H                 (Mt = M^T state, (DK, DV))

(I+L)^{-1} is computed exactly with the nilpotent factorization
(I-L)(I+L^2)(I+L^4)(I+L^8)  [L^16 and beyond are numerically zero here].
Inversion machinery runs in fp16 matmuls (full PE rate, 10-bit mantissa),
state-path matmuls run as float32r (full rate at N>=256).
"""

import numpy as np

import concourse.bass as bass
import concourse.mybir as mybir
import concourse.tile as tile
from concourse.bass import ts
from concourse.bass_utils import run_bass_kernel_spmd
from concourse.masks import make_identity

F32 = mybir.dt.float32
F32R = mybir.dt.float32r
F16 = mybir.dt.float16
AOP = mybir.AluOpType
AFT = mybir.ActivationFunctionType

B, S, DK, DV = 16, 4096, 256, 256
NCORES = 8
BLOC = B // NCORES          # batches per core
C = 128                     # chunk length
LR = 0.1
AC = 1.0 + LR               # 1.1
NLEV = 3                    # squaring levels: (I-L)(I+L^2)(I+L^4)(I+L^8)


def _split_waits(nc, max_waits=1):
    """walrus codegen on this toolchain encodes at most one semaphore wait per
    instruction; hoist excess waits onto same-engine NoOps placed just before."""
    n_split = 0
    for f in nc.m.functions:
        for bb in f.blocks:
            insts = bb.instructions
            out = []
            for inst in insts:
                si = getattr(inst, "sync_info", None)
                w = list(si.on_wait) if (si and si.on_wait) else []
                k = 0
                while len(w) > max_waits:
                    head, w = w[:max_waits], w[max_waits:]
                    out.append(mybir.InstNoOp(
                        name=f"{inst.name}-wsplit{k}",
                        engine=inst.engine,
                        sync_info=mybir.SyncInfo(on_wait=head, on_update=[]),
                    ))
                    n_split += 1
                    k += 1
                if k:
                    inst.sync_info = mybir.SyncInfo(
                        on_wait=w, on_update=list(si.on_update or [])
                    )
                out.append(inst)
            bb.instructions = out
    return n_split


def build_nc(s_loc=S, state_mm_dtype=F32R, split=True):
    nch = s_loc // C
    nc = bass.Bass()
    memT = nc.declare_dram_parameter("memT", [BLOC, DK, DV], F32, isOutput=False)
    key_d = nc.declare_dram_parameter("key", [BLOC, s_loc, DK], F32, isOutput=False)
    val_d = nc.declare_dram_parameter("value", [BLOC, s_loc, DV], F32, isOutput=False)
    outT = nc.declare_dram_parameter("outT", [BLOC, DK, DV], F32, isOutput=True)

    SMM = state_mm_dtype  # state-path matmul tiles (float32r: full-rate fp32-ish mm)

    with tile.TileContext(nc) as tc:
        with (
            tc.tile_pool(name="consts", bufs=1) as consts,
            tc.tile_pool(name="kv", bufs=4) as kv,
            tc.tile_pool(name="norm", bufs=4) as normp,
            tc.tile_pool(name="kt", bufs=3) as ktp,
            tc.tile_pool(name="inv", bufs=3) as invp,
            tc.tile_pool(name="state", bufs=3) as statep,
            tc.tile_pool(name="mt", bufs=3) as mtp,
            tc.tile_pool(name="ps_inv", bufs=3, space="PSUM") as ps_inv,
            tc.tile_pool(name="ps_state", bufs=3, space="PSUM") as ps_state,
            tc.tile_pool(name="ps_mt0", bufs=1, space="PSUM") as ps_mt0,
            tc.tile_pool(name="ps_mt1", bufs=1, space="PSUM") as ps_mt1,
        ):
            ident32 = consts.tile([128, 128], F32, tag="ident32")
            make_identity(nc, ident32)
            ident16 = consts.tile([128, 128], F16, tag="ident16")
            make_identity(nc, ident16)
            # paired identity (both halves) for G0 = I + LTn
            i2_16 = consts.tile([128, 2, 128], F16, tag="i2_16")
            nc.gpsimd.memset(i2_16, 0.0)
            nc.gpsimd.affine_select(
                out=i2_16, in_=i2_16, compare_op=AOP.not_equal, fill=1.0,
                base=0, pattern=[[0, 2], [-1, 128]], channel_multiplier=1,
            )

            # state Mt (= M^T) per batch lives in PSUM and accumulates the
            # per-chunk updates; an SBUF f32r copy is refreshed each chunk.
            # Initial value injected via exact fp32 identity-matmul.
            mt = []
            mt_ps = []
            for b, pool in ((0, ps_mt0), (1, ps_mt1)):
                t0 = mtp.tile([128, 2, DV], F32, tag=f"mt0f{b}")
                nc.sync.dma_start(
                    out=t0, in_=memT[b].rearrange("(j p) v -> p j v", p=128)
                )
                ps = pool.tile([128, 2, DV], F32, tag=f"mtps{b}")
                # one matmul over the whole [128, 512] bank: a second
                # start=True would clear the first slice's has_written bits
                nc.tensor.matmul(ps.rearrange("p j v -> p (j v)"), ident32,
                                 t0.rearrange("p j v -> p (j v)"),
                                 start=True, stop=False,
                                 skip_group_check=True)
                t = mtp.tile([128, 2, DV], SMM, tag=f"mt{b}")
                nc.vector.tensor_copy(t, ps)
                mt.append(t)
                mt_ps.append(ps)

            def cp(dst, src_ap, b, scale=None):
                """psum->sbuf copy of one batch slice; b0 -> DVE, b1 -> ACT."""
                if b == 0:
                    if scale is None:
                        nc.vector.tensor_copy(dst, src_ap)
                    else:
                        nc.vector.tensor_scalar_mul(dst, src_ap, scale)
                else:
                    if scale is None:
                        nc.scalar.copy(dst, src_ap)
                    else:
                        nc.scalar.mul(dst, src_ap, scale)

            def emit_precomp(c):
                Kt, Vt, Kn = [], [], []
                for b in range(BLOC):
                    k = kv.tile([128, DK], F32, tag=f"k{b}")
                    nc.sync.dma_start(out=k, in_=key_d[b, c * C:(c + 1) * C, :])
                    v = kv.tile([128, DV], F32, tag=f"v{b}")
                    nc.sync.dma_start(out=v, in_=val_d[b, c * C:(c + 1) * C, :])
                    Kt.append(k)
                    Vt.append(v)
                # per-batch normalization chains (ACT then DVE / ACT)
                for b in range(BLOC):
                    scr = normp.tile([128, DK], F32, tag="scr")
                    ssq = normp.tile([128, 1], F32, tag=f"ssq{b}")
                    nc.scalar.activation(out=scr, in_=Kt[b], func=AFT.Square,
                                         accum_out=ssq)
                    nrm = normp.tile([128, 1], F32, tag=f"nrm{b}")
                    nc.scalar.activation(nrm, ssq, AFT.Sqrt)
                    rn = normp.tile([128, 1], F32, tag=f"rn{b}")
                    nc.vector.reciprocal(rn, nrm)
                    kn = normp.tile([128, DK], SMM, tag=f"kn{b}")
                    if b == 0:
                        nc.vector.tensor_scalar_mul(kn, Kt[b], rn)
                    else:
                        nc.scalar.activation(kn, Kt[b], AFT.Copy, scale=rn)
                    Kn.append(kn)

                # transposes of Kn -> KnTs (f32r; feeds A-mm and Y-mm)
                KnTs = [None] * 2
                for j in range(2):
                    tp = ps_inv.tile([128, 2, 128], F32, tag="inv")
                    for b in range(BLOC):
                        nc.tensor.transpose(
                            tp[:, b, :], Kn[b][:, ts(j, 128)].bitcast(F32),
                            ident32)
                    s32 = ktp.tile([128, 2, 128], SMM, tag=f"knts{j}")
                    for b in range(BLOC):
                        cp(s32[:, b, :], tp[:, b, :], b)
                    KnTs[j] = s32

                # A = Kn Kn^T (f32r matmul; batches share the psum tile)
                a_ps = ps_inv.tile([128, 2, 128], F32, tag="inv")
                for b in range(BLOC):
                    for j in range(2):
                        nc.tensor.matmul(
                            a_ps[:, b, :], KnTs[j][:, b, :], KnTs[j][:, b, :],
                            start=(j == 0), stop=(j == 1),
                        )
                a_neg = invp.tile([128, 2, 128], F16, tag="a_neg")
                for b in range(BLOC):
                    cp(a_neg[:, b, :], a_ps[:, b, :], b, scale=-AC)
                # Ln = -L = strict_lower(a_neg); LTn = strict_upper (per batch)
                ln = invp.tile([128, 2, 128], F16, tag="ln")
                ltn = invp.tile([128, 2, 128], F16, tag="ltn")
                for b in range(BLOC):
                    nc.gpsimd.affine_select(
                        out=ln[:, b, :], in_=a_neg[:, b, :],
                        compare_op=AOP.is_gt, fill=0.0,
                        base=0, pattern=[[-1, 128]], channel_multiplier=1,
                    )
                    nc.gpsimd.affine_select(
                        out=ltn[:, b, :], in_=a_neg[:, b, :],
                        compare_op=AOP.is_gt, fill=0.0,
                        base=0, pattern=[[1, 128]], channel_multiplier=-1,
                    )

                # power chain; L/LT pairs packed into one [128, 2, 256] psum
                def pow_pair(lhsT_l, rhs_l, lhsT_lt, rhs_lt, tag):
                    ps = ps_inv.tile([128, 2, 256], F32, tag="inv")
                    for b in range(BLOC):
                        nc.tensor.matmul(ps[:, b, 0:128],
                                         lhsT_l[:, b, :], rhs_l[:, b, :])
                        nc.tensor.matmul(ps[:, b, 128:256],
                                         lhsT_lt[:, b, :], rhs_lt[:, b, :])
                    sb = invp.tile([128, 2, 256], F16, tag=tag)
                    for b in range(BLOC):
                        cp(sb[:, b, :], ps[:, b, :], b)
                    return sb[:, :, 0:128], sb[:, :, 128:256]

                l2, lt2 = pow_pair(ltn, ln, ln, ltn, "p2")
                l4, lt4 = pow_pair(lt2, l2, l2, lt2, "p4")
                p8 = ps_inv.tile([128, 2, 128], F32, tag="inv")
                for b in range(BLOC):
                    nc.tensor.matmul(p8[:, b, :], lt4[:, b, :], l4[:, b, :])
                l8 = invp.tile([128, 2, 128], F16, tag="p8")
                for b in range(BLOC):
                    cp(l8[:, b, :], p8[:, b, :], b)

                # G chain: G0 = I + LTn; G <- (I + LT^{2^i}) G
                g = invp.tile([128, 2, 128], F16, tag="g0")
                for b in range(BLOC):
                    nc.vector.tensor_add(g[:, b, :], i2_16[:, b, :],
                                         ltn[:, b, :])
                for i, lp in enumerate((l2, l4, l8)):
                    gp = ps_inv.tile([128, 2, 128], F32, tag="inv")
                    gn = invp.tile([128, 2, 128], F16, tag=f"g{i + 1}")
                    if i == 1:
                        # G' = G + LT^4 G : matmul + DVE add (no inject)
                        for b in range(BLOC):
                            nc.tensor.matmul(gp[:, b, :], lp[:, b, :],
                                             g[:, b, :])
                        for b in range(BLOC):
                            nc.vector.tensor_add(gn[:, b, :], g[:, b, :],
                                                 gp[:, b, :])
                    else:
                        for b in range(BLOC):
                            nc.tensor.matmul(gp[:, b, :], lp[:, b, :],
                                             g[:, b, :],
                                             start=True, stop=False)
                            nc.tensor.matmul(gp[:, b, :], ident16, g[:, b, :],
                                             start=False, stop=True)
                        for b in range(BLOC):
                            cp(gn[:, b, :], gp[:, b, :], b)
                    g = gn
                return dict(Kn=Kn, Vt=Vt, KnTs=KnTs, g=g, c=c)

            def emit_state(art):
                Kn, Vt, KnTs, g = art["Kn"], art["Vt"], art["KnTs"], art["g"]
                last = art["c"] == nch - 1
                y_ps, rh, h_ps, h_sb = [], [], [], []
                for b in range(BLOC):
                    y = ps_state.tile([128, DV], F32, tag="st")
                    for j in range(2):
                        nc.tensor.matmul(
                            y, KnTs[j][:, b, :], mt[b][:, j, :],
                            start=(j == 0), stop=(j == 1),
                        )
                    y_ps.append(y)
                for b in range(BLOC):
                    # R' = 10*R = -11 Kn Mt + V  (fp16); the 0.1 folds into H
                    r = statep.tile([128, DV], F16, tag=f"rh{b}")
                    nc.vector.scalar_tensor_tensor(
                        out=r, in0=y_ps[b], scalar=-10.0 * AC, in1=Vt[b],
                        op0=AOP.mult, op1=AOP.add,
                    )
                    rh.append(r)
                for b in range(BLOC):
                    h = ps_state.tile([128, DV], F32, tag="st")
                    nc.tensor.matmul(h, g[:, b, :], rh[b])
                    h_ps.append(h)
                for b in range(BLOC):
                    h = statep.tile([128, DV], SMM, tag=f"hs{b}")
                    cp(h, h_ps[b], b, scale=LR)        # H = 0.1 * Tinv R'
                    h_sb.append(h)
                for b in range(BLOC):
                    for j in range(2):
                        nc.tensor.matmul(
                            mt_ps[b][:, j, :], Kn[b][:, ts(j, 128)], h_sb[b],
                            start=False, stop=last, skip_group_check=True,
                        )
                for b in range(BLOC):
                    mt_new = mtp.tile([128, 2, DV], SMM, tag=f"mt{b}")
                    cp(mt_new, mt_ps[b], b)
                    mt[b] = mt_new

            # software pipeline: chunk c+1's state-independent precompute is
            # emitted before chunk c's state path so the PE always has
            # independent work while psum->sbuf copies drain.
            arts = [emit_precomp(0), emit_precomp(1) if nch > 1 else None]
            for c in range(nch):
                nxt = emit_precomp(c + 2) if c + 2 < nch else None
                emit_state(arts[0])
                arts = [arts[1], nxt]

            for b in range(BLOC):
                nc.sync.dma_start(
                    out=outT[b].rearrange("(j p) v -> p j v", p=128),
                    in_=mt[b].bitcast(F32),
                )
    if split:
        _split_waits(nc)
    return nc


_NC_CACHE = {}

# test-harness hooks (the grading harness just calls kernel())
TRACE = False
LAST_RESULT = None


def _get_nc(s_loc=S):
    if s_loc not in _NC_CACHE:
        _NC_CACHE[s_loc] = build_nc(s_loc)
    return _NC_CACHE[s_loc]


def kernel(memory, key, value):
    global LAST_RESULT
    memory = np.ascontiguousarray(np.asarray(memory), dtype=np.float32)
    key = np.ascontiguousarray(np.asarray(key), dtype=np.float32)
    value = np.ascontiguousarray(np.asarray(value), dtype=np.float32)
    s_loc = key.shape[1]
    nc = _get_nc(s_loc)
    memT = np.ascontiguousarray(memory.transpose(0, 2, 1))
    in_maps = []
    for i in range(NCORES):
        sl = slice(i * BLOC, (i + 1) * BLOC)
        in_maps.append({
            "memT": memT[sl],
            "key": np.ascontiguousarray(key[sl]),
            "value": np.ascontiguousarray(value[sl]),
        })
    res = run_bass_kernel_spmd(nc, in_maps, list(range(NCORES)), trace=TRACE)
    LAST_RESULT = res
    outs = [res.results[i]["outT"] for i in range(NCORES)]
    out = np.concatenate(outs, axis=0)          # (16, DK, DV) = M^T
    return np.ascontiguousarray(out.transpose(0, 2, 1))


# revision 21
# speedup vs baseline: 1.5197x; 1.4432x over previous
"""Trainium2 Bass kernel for the delta-rule memory recurrence (DeltaNet-style).

Full-input contract: kernel(memory, key, value) -> final memory, all np.ndarray,
shapes (16,256,256), (16,4096,256), (16,4096,256) -> (16,256,256) float32.

Strategy: pure data-parallel over batch (2 batches per NeuronCore x 8 cores).
Per batch the sequential recurrence

    kn   = k_t / ||k_t||
    M   <- M - (1.1 * M kn - 0.1 * v_t) kn^T

is reformulated chunkwise (C=128 steps per chunk) via the WY / UT transform:

    A  = Kn Kn^T                      (C x C Gram of normalized keys)
    L  = 1.1 * strict_lower(A)
    Tinv = (I + L)^{-1}               (unit lower triangular inverse)
    H  = Tinv @ (-1.1 * Kn Mt + 0.1 * V)
    Mt <- Mt + Kn^T H                 (Mt = M^T state, (DK, DV))

(I+L)^{-1} is computed exactly with the nilpotent factorization
(I-L)(I+L^2)(I+L^4)(I+L^8)  [L^16 and beyond are numerically zero here].
Inversion machinery runs in fp16 matmuls (full PE rate, 10-bit mantissa),
state-path matmuls run as float32r (full rate at N>=256).
"""

import numpy as np

import concourse.bass as bass
import concourse.mybir as mybir
import concourse.tile as tile
from concourse.bass import ts
from concourse.bass_utils import run_bass_kernel_spmd
from concourse.masks import make_identity

F32 = mybir.dt.float32
F32R = mybir.dt.float32r
F16 = mybir.dt.float16
AOP = mybir.AluOpType
AFT = mybir.ActivationFunctionType

B, S, DK, DV = 16, 4096, 256, 256
NCORES = 8
BLOC = B // NCORES          # batches per core
C = 128                     # chunk length
LR = 0.1
AC = 1.0 + LR               # 1.1
NLEV = 3                    # squaring levels: (I-L)(I+L^2)(I+L^4)(I+L^8)


def _split_waits(nc, max_waits=1):
    """walrus codegen on this toolchain encodes at most one semaphore wait per
    instruction; hoist excess waits onto same-engine NoOps placed just before."""
    n_split = 0
    for f in nc.m.functions:
        for bb in f.blocks:
            insts = bb.instructions
            out = []
            for inst in insts:
                si = getattr(inst, "sync_info", None)
                w = list(si.on_wait) if (si and si.on_wait) else []
                k = 0
                while len(w) > max_waits:
                    head, w = w[:max_waits], w[max_waits:]
                    out.append(mybir.InstNoOp(
                        name=f"{inst.name}-wsplit{k}",
                        engine=inst.engine,
                        sync_info=mybir.SyncInfo(on_wait=head, on_update=[]),
                    ))
                    n_split += 1
                    k += 1
                if k:
                    inst.sync_info = mybir.SyncInfo(
                        on_wait=w, on_update=list(si.on_update or [])
                    )
                out.append(inst)
            bb.instructions = out
    return n_split


def build_nc(s_loc=S, state_mm_dtype=F32R, split=True):
    nch = s_loc // C
    nc = bass.Bass()
    memT = nc.declare_dram_parameter("memT", [BLOC, DK, DV], F32, isOutput=False)
    key_d = nc.declare_dram_parameter("key", [BLOC, s_loc, DK], F32, isOutput=False)
    val_d = nc.declare_dram_parameter("value", [BLOC, s_loc, DV], F32, isOutput=False)
    outT = nc.declare_dram_parameter("outT", [BLOC, DK, DV], F32, isOutput=True)

    SMM = state_mm_dtype  # state-path matmul tiles (float32r: full-rate fp32-ish mm)

    with tile.TileContext(nc) as tc:
        with (
            tc.tile_pool(name="consts", bufs=1) as consts,
            tc.tile_pool(name="kv", bufs=10) as kv,
            tc.tile_pool(name="norm", bufs=10) as normp,
            tc.tile_pool(name="kt", bufs=10) as ktp,
            tc.tile_pool(name="inv", bufs=6) as invp,
            tc.tile_pool(name="state", bufs=3) as statep,
            tc.tile_pool(name="mt", bufs=3) as mtp,
            tc.tile_pool(name="ps_inv", bufs=4, space="PSUM") as ps_inv,
            tc.tile_pool(name="ps_state", bufs=2, space="PSUM") as ps_state,
            tc.tile_pool(name="ps_mt0", bufs=1, space="PSUM") as ps_mt0,
            tc.tile_pool(name="ps_mt1", bufs=1, space="PSUM") as ps_mt1,
        ):
            ident32 = consts.tile([128, 128], F32, tag="ident32")
            make_identity(nc, ident32)
            ident16 = consts.tile([128, 128], F16, tag="ident16")
            make_identity(nc, ident16)
            # paired identity (both halves) for G0 = I + LTn
            i2_16 = consts.tile([128, 2, 128], F16, tag="i2_16")
            nc.gpsimd.memset(i2_16, 0.0)
            nc.gpsimd.affine_select(
                out=i2_16, in_=i2_16, compare_op=AOP.not_equal, fill=1.0,
                base=0, pattern=[[0, 2], [-1, 128]], channel_multiplier=1,
            )

            # state Mt (= M^T) per batch lives in PSUM and accumulates the
            # per-chunk updates; an SBUF f32r copy is refreshed each chunk.
            # Initial value injected via exact fp32 identity-matmul.
            mt = []
            mt_ps = []
            for b, pool in ((0, ps_mt0), (1, ps_mt1)):
                t0 = mtp.tile([128, 2, DV], F32, tag=f"mt0f{b}")
                nc.sync.dma_start(
                    out=t0, in_=memT[b].rearrange("(j p) v -> p j v", p=128)
                )
                ps = pool.tile([128, 2, DV], F32, tag=f"mtps{b}")
                # one matmul over the whole [128, 512] bank: a second
                # start=True would clear the first slice's has_written bits
                nc.tensor.matmul(ps.rearrange("p j v -> p (j v)"), ident32,
                                 t0.rearrange("p j v -> p (j v)"),
                                 start=True, stop=False,
                                 skip_group_check=True)
                t = mtp.tile([128, 2, DV], SMM, tag=f"mt{b}")
                nc.vector.tensor_copy(t, ps)
                mt.append(t)
                mt_ps.append(ps)

            def cp(dst, src_ap, b, scale=None):
                """psum->sbuf copy of one batch slice; b0 -> DVE, b1 -> ACT."""
                if b == 0:
                    if scale is None:
                        nc.vector.tensor_copy(dst, src_ap)
                    else:
                        nc.vector.tensor_scalar_mul(dst, src_ap, scale)
                else:
                    if scale is None:
                        nc.scalar.copy(dst, src_ap)
                    else:
                        nc.scalar.mul(dst, src_ap, scale)

            def emit_precomp_batch(cs):
                """Stage-major precompute for several chunks: each stage runs
                across all chunks back-to-back so the PE stream stays dense."""
                A = [dict(c=c) for c in cs]
                for a in A:                       # loads
                    c = a["c"]
                    a["Kt"], a["Vt"] = [], []
                    for b in range(BLOC):
                        k = kv.tile([128, DK], F32, tag=f"k{b}")
                        nc.sync.dma_start(out=k,
                                          in_=key_d[b, c * C:(c + 1) * C, :])
                        v = kv.tile([128, DV], F32, tag=f"v{b}")
                        nc.sync.dma_start(out=v,
                                          in_=val_d[b, c * C:(c + 1) * C, :])
                        a["Kt"].append(k)
                        a["Vt"].append(v)
                for a in A:                       # normalization
                    a["Kn"] = []
                    for b in range(BLOC):
                        scr = normp.tile([128, DK], F32, tag="scr")
                        ssq = normp.tile([128, 1], F32, tag=f"ssq{b}")
                        nc.scalar.activation(out=scr, in_=a["Kt"][b],
                                             func=AFT.Square, accum_out=ssq)
                        nrm = normp.tile([128, 1], F32, tag=f"nrm{b}")
                        nc.scalar.activation(nrm, ssq, AFT.Sqrt)
                        rn = normp.tile([128, 1], F32, tag=f"rn{b}")
                        nc.vector.reciprocal(rn, nrm)
                        kn = normp.tile([128, DK], SMM, tag=f"kn{b}")
                        if b == 0:
                            nc.vector.tensor_scalar_mul(kn, a["Kt"][b], rn)
                        else:
                            nc.scalar.activation(kn, a["Kt"][b], AFT.Copy,
                                                 scale=rn)
                        a["Kn"].append(kn)
                for a in A:                       # transposes
                    a["KnTs"] = []
                    for j in range(2):
                        tp = ps_inv.tile([128, 2, 128], F32, tag="inv")
                        for b in range(BLOC):
                            nc.tensor.transpose(
                                tp[:, b, :],
                                a["Kn"][b][:, ts(j, 128)].bitcast(F32),
                                ident32)
                        s32 = ktp.tile([128, 2, 128], SMM, tag=f"knts{j}")
                        for b in range(BLOC):
                            cp(s32[:, b, :], tp[:, b, :], b)
                        a["KnTs"].append(s32)
                for a in A:                       # Gram matrix + masks
                    a_ps = ps_inv.tile([128, 2, 128], F32, tag="inv")
                    for b in range(BLOC):
                        for j in range(2):
                            nc.tensor.matmul(
                                a_ps[:, b, :], a["KnTs"][j][:, b, :],
                                a["KnTs"][j][:, b, :],
                                start=(j == 0), stop=(j == 1),
                            )
                    a_neg = invp.tile([128, 2, 128], F16, tag="a_neg")
                    for b in range(BLOC):
                        cp(a_neg[:, b, :], a_ps[:, b, :], b, scale=-AC)
                    a["a_neg"] = a_neg
                for a in A:                       # triangular masks (gpsimd)
                    ln = invp.tile([128, 2, 128], F16, tag="ln")
                    ltn = invp.tile([128, 2, 128], F16, tag="ltn")
                    for b in range(BLOC):
                        nc.gpsimd.affine_select(
                            out=ln[:, b, :], in_=a["a_neg"][:, b, :],
                            compare_op=AOP.is_gt, fill=0.0,
                            base=0, pattern=[[-1, 128]], channel_multiplier=1,
                        )
                        nc.gpsimd.affine_select(
                            out=ltn[:, b, :], in_=a["a_neg"][:, b, :],
                            compare_op=AOP.is_gt, fill=0.0,
                            base=0, pattern=[[1, 128]], channel_multiplier=-1,
                        )
                    a["ln"], a["ltn"] = ln, ltn

                def pow_pair(a, lhsT_l, rhs_l, lhsT_lt, rhs_lt, tag):
                    ps = ps_inv.tile([128, 2, 256], F32, tag="inv")
                    for b in range(BLOC):
                        nc.tensor.matmul(ps[:, b, 0:128],
                                         lhsT_l[:, b, :], rhs_l[:, b, :])
                        nc.tensor.matmul(ps[:, b, 128:256],
                                         lhsT_lt[:, b, :], rhs_lt[:, b, :])
                    sb = invp.tile([128, 2, 256], F16, tag=tag)
                    for b in range(BLOC):
                        cp(sb[:, b, :], ps[:, b, :], b)
                    return sb[:, :, 0:128], sb[:, :, 128:256]

                for a in A:                       # L^2 / L^2T
                    a["l2"], a["lt2"] = pow_pair(a, a["ltn"], a["ln"],
                                                 a["ln"], a["ltn"], "p2")
                for a in A:                       # G0 (while pow4 brews)
                    g = invp.tile([128, 2, 128], F16, tag="g0")
                    for b in range(BLOC):
                        nc.vector.tensor_add(g[:, b, :], i2_16[:, b, :],
                                             a["ltn"][:, b, :])
                    a["g"] = g
                for a in A:                       # L^4 / L^4T
                    a["l4"], a["lt4"] = pow_pair(a, a["lt2"], a["l2"],
                                                 a["l2"], a["lt2"], "p4")
                for a in A:                       # G1 = (I + LT2) G0
                    gp = ps_inv.tile([128, 2, 128], F32, tag="inv")
                    gn = invp.tile([128, 2, 128], F16, tag="g1")
                    for b in range(BLOC):
                        nc.tensor.matmul(gp[:, b, :], a["l2"][:, b, :],
                                         a["g"][:, b, :],
                                         start=True, stop=False)
                        nc.tensor.matmul(gp[:, b, :], ident16, a["g"][:, b, :],
                                         start=False, stop=True)
                    for b in range(BLOC):
                        cp(gn[:, b, :], gp[:, b, :], b)
                    a["g"] = gn
                for a in A:                       # L^8
                    p8 = ps_inv.tile([128, 2, 128], F32, tag="inv")
                    for b in range(BLOC):
                        nc.tensor.matmul(p8[:, b, :], a["lt4"][:, b, :],
                                         a["l4"][:, b, :])
                    l8 = invp.tile([128, 2, 128], F16, tag="p8")
                    for b in range(BLOC):
                        cp(l8[:, b, :], p8[:, b, :], b)
                    a["l8"] = l8
                for a in A:                       # G2 = (I + LT4) G1 (DVE add)
                    gp = ps_inv.tile([128, 2, 128], F32, tag="inv")
                    gn = invp.tile([128, 2, 128], F16, tag="g2")
                    for b in range(BLOC):
                        nc.tensor.matmul(gp[:, b, :], a["l4"][:, b, :],
                                         a["g"][:, b, :])
                    for b in range(BLOC):
                        nc.vector.tensor_add(gn[:, b, :], a["g"][:, b, :],
                                             gp[:, b, :])
                    a["g"] = gn
                for a in A:                       # G3 = (I + LT8) G2
                    gp = ps_inv.tile([128, 2, 128], F32, tag="inv")
                    gn = invp.tile([128, 2, 128], F16, tag="g3")
                    for b in range(BLOC):
                        nc.tensor.matmul(gp[:, b, :], a["l8"][:, b, :],
                                         a["g"][:, b, :],
                                         start=True, stop=False)
                        nc.tensor.matmul(gp[:, b, :], ident16, a["g"][:, b, :],
                                         start=False, stop=True)
                    for b in range(BLOC):
                        cp(gn[:, b, :], gp[:, b, :], b)
                    a["g"] = gn
                return A

            def emit_state(art):
                Kn, Vt, KnTs, g = art["Kn"], art["Vt"], art["KnTs"], art["g"]
                last = art["c"] == nch - 1
                y_ps, rh, h_ps, h_sb = [], [], [], []
                for b in range(BLOC):
                    y = ps_state.tile([128, DV], F32, tag="st")
                    for j in range(2):
                        nc.tensor.matmul(
                            y, KnTs[j][:, b, :], mt[b][:, j, :],
                            start=(j == 0), stop=(j == 1),
                        )
                    y_ps.append(y)
                for b in range(BLOC):
                    # R' = 10*R = -11 Kn Mt + V  (fp16); the 0.1 folds into H
                    r = statep.tile([128, DV], F16, tag=f"rh{b}")
                    nc.vector.scalar_tensor_tensor(
                        out=r, in0=y_ps[b], scalar=-10.0 * AC, in1=Vt[b],
                        op0=AOP.mult, op1=AOP.add,
                    )
                    rh.append(r)
                for b in range(BLOC):
                    h = ps_state.tile([128, DV], F32, tag="st")
                    nc.tensor.matmul(h, g[:, b, :], rh[b])
                    h_ps.append(h)
                for b in range(BLOC):
                    h = statep.tile([128, DV], SMM, tag=f"hs{b}")
                    cp(h, h_ps[b], b, scale=LR)        # H = 0.1 * Tinv R'
                    h_sb.append(h)
                for b in range(BLOC):
                    for j in range(2):
                        nc.tensor.matmul(
                            mt_ps[b][:, j, :], Kn[b][:, ts(j, 128)], h_sb[b],
                            start=False, stop=last, skip_group_check=True,
                        )
                for b in range(BLOC):
                    mt_new = mtp.tile([128, 2, DV], SMM, tag=f"mt{b}")
                    cp(mt_new, mt_ps[b], b)
                    mt[b] = mt_new

            # software pipeline, super-batched: the state-independent
            # precompute for the NEXT group of chunks is emitted stage-major
            # (dense independent matmul streams) before this group's
            # sequential state chain.
            SB = 4
            groups = [list(range(i, min(i + SB, nch))) for i in range(0, nch, SB)]
            arts = emit_precomp_batch(groups[0])
            for gi, grp in enumerate(groups):
                nxt = (emit_precomp_batch(groups[gi + 1])
                       if gi + 1 < len(groups) else None)
                for art in arts:
                    emit_state(art)
                arts = nxt

            for b in range(BLOC):
                nc.sync.dma_start(
                    out=outT[b].rearrange("(j p) v -> p j v", p=128),
                    in_=mt[b].bitcast(F32),
                )
    if split:
        _split_waits(nc)
    return nc


_NC_CACHE = {}

# test-harness hooks (the grading harness just calls kernel())
TRACE = False
LAST_RESULT = None


def _get_nc(s_loc=S):
    if s_loc not in _NC_CACHE:
        _NC_CACHE[s_loc] = build_nc(s_loc)
    return _NC_CACHE[s_loc]


def kernel(memory, key, value):
    global LAST_RESULT
    memory = np.ascontiguousarray(np.asarray(memory), dtype=np.float32)
    key = np.ascontiguousarray(np.asarray(key), dtype=np.float32)
    value = np.ascontiguousarray(np.asarray(value), dtype=np.float32)
    s_loc = key.shape[1]
    nc = _get_nc(s_loc)
    memT = np.ascontiguousarray(memory.transpose(0, 2, 1))
    in_maps = []
    for i in range(NCORES):
        sl = slice(i * BLOC, (i + 1) * BLOC)
        in_maps.append({
            "memT": memT[sl],
            "key": np.ascontiguousarray(key[sl]),
            "value": np.ascontiguousarray(value[sl]),
        })
    res = run_bass_kernel_spmd(nc, in_maps, list(range(NCORES)), trace=TRACE)
    LAST_RESULT = res
    outs = [res.results[i]["outT"] for i in range(NCORES)]
    out = np.concatenate(outs, axis=0)          # (16, DK, DV) = M^T
    return np.ascontiguousarray(out.transpose(0, 2, 1))
